# revision 8
# baseline (speedup 1.0000x reference)
# Trainium2 Bass kernel for nn_ConditionedCTKoopmanTransition.
#
# Math (reference): z' = z @ A_bar^T + u @ B_bar^T ; y = z' @ C^T + (u*dt) @ D^T
# scanned over T=256 steps, with A_bar = expm(A_ct*dt), B_bar = A^-1 (A_bar-I) B_ct
# built host-side in float64 from the tiny parameter tensors.
#
# Strategy: data-parallel over batch (8 cores x 64 batch). On each core the
# T=256 sequential scan is restructured into 8 chunks of S=32 steps:
#   F_k   = G @ ublk_k            (G = [A^31 B | ... | B], one K=1024 matmul)
#   a_k+1 = A^32 @ a_k + F_k      (tiny 7-step sequential chain)
#   rollout all 8 chunks batched in the matmul free dim (N=512) for 32 steps
# so nearly all tensor-engine work runs as [K=128, M=128, N=256] fp32r
# matmuls at full (1 cycle/row) speed instead of 256 sequential N=64 steps.
# State is kept d-major (z^T) so each step's PSUM output is directly the
# next step's matmul rhs -- no transposes anywhere on device.

import sys
import numpy as np

sys.path.insert(0, "/opt/trn_rl_repo")

D = 512
UD = 32
NOBS = 50
BATCH = 512
T = 256
NCORES = 8
BS = BATCH // NCORES      # batch shard per core = 64
S = 32                    # chunk length
NCH = T // S              # chunks = 8
HALF = 256                # free-dim half (4 chunks * 64 batch)

_PROGRAM_CACHE = {}
TRACE = False             # test harness can set kernel.TRACE = True
LAST_RESULT = None        # BassKernelResults of the last run (when TRACE)


def _softplus64(x):
    x = np.asarray(x, np.float64)
    return np.log1p(np.exp(-np.abs(x))) + np.maximum(x, 0.0)


def _host_precompute(dt_val, A_skew_params, gamma_raw, B_ct, C, D_mat):
    """float64 host math for the small matrices."""
    import scipy.linalg as sla
    d = D
    A = np.zeros((d, d), np.float64)
    iu = np.triu_indices(d, k=1)
    A[iu] = np.asarray(A_skew_params, np.float64)
    A = A - A.T
    A_ct = A - np.diag(_softplus64(gamma_raw))
    A_bar = sla.expm(A_ct * float(dt_val))
    B_bar = np.linalg.solve(A_ct, (A_bar - np.eye(d)) @ np.asarray(B_ct, np.float64))
    G = np.zeros((d, S * UD), np.float64)
    M = B_bar.copy()
    for j in range(S - 1, -1, -1):
        G[:, j * UD:(j + 1) * UD] = M
        if j > 0:
            M = A_bar @ M
    A_S = np.linalg.matrix_power(A_bar, S)
    f32 = np.float32
    return {
        "WAT": np.ascontiguousarray(A_bar.T.astype(f32)),          # [512, 512] lhsT for A_bar
        "AST": np.ascontiguousarray(A_S.T.astype(f32)),            # [512, 512] lhsT for A^S
        "GT": np.ascontiguousarray(G.T.astype(f32)),               # [1024, 512] lhsT for G
        "WBT": np.ascontiguousarray(np.tile(B_bar.T.astype(f32), (2, 1))),   # [64, 512]
        "WCT": np.ascontiguousarray(np.asarray(C, np.float64).T.astype(f32)),    # [512, 50]
        "WDT": np.ascontiguousarray(np.tile((np.asarray(D_mat, np.float64) * float(dt_val)).T.astype(f32), (2, 1))),  # [64, 50]
    }


def _build_program():
    from concourse import bacc, tile, mybir

    f32 = mybir.dt.float32
    f32r = mybir.dt.float32r

    nc = bacc.Bacc("TRN2", target_bir_lowering=False, debug=False,
                   num_devices=NCORES)

    # DRAM I/O (all fp32 bits; float32r dtype where consumed by matmuls so
    # plain DMAs land in float32r tiles without a casting engine).
    wat_d = nc.dram_tensor("wat", [D, D], f32r, kind="ExternalInput")
    ast_d = nc.dram_tensor("ast", [D, D], f32r, kind="ExternalInput")
    gt_d = nc.dram_tensor("gt", [S * UD, D], f32r, kind="ExternalInput")
    wbt_d = nc.dram_tensor("wbt", [64, D], f32r, kind="ExternalInput")
    wct_d = nc.dram_tensor("wct", [D, NOBS], f32r, kind="ExternalInput")
    wdt_d = nc.dram_tensor("wdt", [64, NOBS], f32r, kind="ExternalInput")
    uall_d = nc.dram_tensor("uall", [S * UD, NCH * BS], f32r, kind="ExternalInput")
    zt0_d = nc.dram_tensor("zt0", [D, BS], f32r, kind="ExternalInput")
    ztout_d = nc.dram_tensor("ztout", [NCH, S, D, BS], f32r, kind="ExternalOutput")
    ytout_d = nc.dram_tensor("ytout", [NCH, S, NOBS, BS], f32, kind="ExternalOutput")

    KT = D // 128   # 4 k-tiles of the d dimension

    with tile.TileContext(nc) as tc:
        with tc.tile_pool(name="const", bufs=1) as cpool, \
             tc.tile_pool(name="fsb", bufs=1) as fpool, \
             tc.tile_pool(name="anp", bufs=1) as anpool, \
             tc.tile_pool(name="st", bufs=2) as stpool, \
             tc.tile_pool(name="ysb", bufs=3) as ypool, \
             tc.tile_pool(name="acc", bufs=5, space="PSUM") as apool, \
             tc.tile_pool(name="yacc", bufs=2, space="PSUM") as yapool:

            # ---- load constants ----
            wa = []
            ast = []
            wc = []
            for kk in range(KT):
                t = cpool.tile([128, D], f32r, tag=f"wa{kk}")
                nc.sync.dma_start(t[:], wat_d.ap()[128 * kk:128 * (kk + 1), :])
                wa.append(t)
                t = cpool.tile([128, D], f32r, tag=f"ast{kk}")
                nc.sync.dma_start(t[:], ast_d.ap()[128 * kk:128 * (kk + 1), :])
                ast.append(t)
                t = cpool.tile([128, NOBS], f32r, tag=f"wc{kk}")
                nc.sync.dma_start(t[:], wct_d.ap()[128 * kk:128 * (kk + 1), :])
                wc.append(t)
            gt = []
            uall = []
            for kk in range(S // 2):   # 16 pair row-tiles of 64 partitions
                t = cpool.tile([64, D], f32r, tag=f"gt{kk}")
                nc.sync.dma_start(t[:], gt_d.ap()[64 * kk:64 * (kk + 1), :])
                gt.append(t)
                t = cpool.tile([64, NCH * BS], f32r, tag=f"u{kk}")
                nc.sync.dma_start(t[:], uall_d.ap()[64 * kk:64 * (kk + 1), :])
                uall.append(t)
            # B/D weights replicated across the 4 partition quarters so the
            # lhsT base_partition matches the u-slice rhs base_partition.
            wb = cpool.tile([64, D], f32r, tag="wb")
            nc.sync.dma_start(wb[:], wbt_d.ap())
            wd = cpool.tile([64, NOBS], f32r, tag="wd")
            nc.sync.dma_start(wd[:], wdt_d.ap())

            def ur_ap(r):
                # u_t^T for step r of every chunk: [32, 512] partition-slice
                q = r % 2
                return uall[r // 2][32 * q:32 * (q + 1), :]

            # ---- phase F: F = G @ UBLK   [512, 512] ----
            fsb = {}
            for h in range(2):
                for m in range(KT):
                    pf = apool.tile([128, HALF], f32, tag="acc")
                    for kk in range(S // 2):
                        nc.tensor.matmul(
                            pf[:],
                            gt[kk][:, 128 * m:128 * (m + 1)],
                            uall[kk][:, HALF * h:HALF * (h + 1)],
                            start=(kk == 0), stop=(kk == S // 2 - 1),
                        )
                    t = fpool.tile([128, HALF], f32, tag=f"f{m}{h}")
                    nc.vector.tensor_copy(t[:], pf[:])
                    fsb[(m, h)] = t

            # ---- anchor chain: a_{k+1} = A^S a_k + F_k ----
            an = {}
            for h in range(2):
                for m in range(KT):
                    an[(m, h)] = anpool.tile([128, HALF], f32r, tag=f"an{m}{h}",
                                             name=f"an{m}{h}")
            for m in range(KT):
                nc.sync.dma_start(an[(m, 0)][:, 0:BS],
                                  zt0_d.ap()[128 * m:128 * (m + 1), :])
            for k in range(NCH - 1):
                hs, cs = k // 4, k % 4
                hd, cd = (k + 1) // 4, (k + 1) % 4
                for m in range(KT):
                    pc = apool.tile([128, BS], f32, tag="acc")
                    for kk in range(KT):
                        nc.tensor.matmul(
                            pc[:],
                            ast[kk][:, 128 * m:128 * (m + 1)],
                            an[(kk, hs)][:, BS * cs:BS * (cs + 1)],
                            start=(kk == 0), stop=(kk == KT - 1),
                        )
                    nc.vector.tensor_tensor(
                        an[(m, hd)][:, BS * cd:BS * (cd + 1)],
                        pc[:],
                        fsb[(m, hs)][:, BS * cs:BS * (cs + 1)],
                        op=mybir.AluOpType.add,
                    )

            # ---- phase C: batched rollout of all chunks ----
            state = {(m, h): an[(m, h)] for m in range(KT) for h in range(2)}
            for r in range(S):
                ur = ur_ap(r)
                for h in range(2):
                    new = {}
                    for m in range(KT):
                        ps = apool.tile([128, HALF], f32, tag="acc")
                        for kk in range(KT):
                            nc.tensor.matmul(
                                ps[:],
                                wa[kk][:, 128 * m:128 * (m + 1)],
                                state[(kk, h)][:],
                                start=(kk == 0), stop=False,
                            )
                        q = r % 2
                        nc.tensor.matmul(
                            ps[:],
                            wb[32 * q:32 * (q + 1), 128 * m:128 * (m + 1)],
                            ur[:, HALF * h:HALF * (h + 1)],
                            start=False, stop=True,
                        )
                        ns = stpool.tile([128, HALF], f32r, tag=f"st{m}{h}")
                        nc.vector.tensor_copy(ns[:], ps[:])
                        new[m] = ns  # noqa
                        # store z' = state after step r for chunks of this half
                        nc.sync.dma_start(
                            ztout_d.ap()[4 * h:4 * (h + 1), r,
                                         128 * m:128 * (m + 1), :]
                            .rearrange("k p e -> p k e"),
                            ns[:].rearrange("p (k e) -> p k e", e=BS),
                        )
                    # y = C z' + (dt D) u
                    py = yapool.tile([NOBS, HALF], f32, tag="yacc")
                    for kk in range(KT):
                        nc.tensor.matmul(
                            py[:], wc[kk][:], new[kk][:],
                            start=(kk == 0), stop=False,
                        )
                    q = r % 2
                    nc.tensor.matmul(
                        py[:], wd[32 * q:32 * (q + 1), :],
                        ur[:, HALF * h:HALF * (h + 1)],
                        start=False, stop=True,
                    )
                    yt = ypool.tile([NOBS, HALF], f32, tag=f"y{h}")
                    nc.vector.tensor_copy(yt[:], py[:])
                    nc.sync.dma_start(
                        ytout_d.ap()[4 * h:4 * (h + 1), r, :, :]
                        .rearrange("k p e -> p k e"),
                        yt[:].rearrange("p (k e) -> p k e", e=BS),
                    )
                    for m in range(KT):
                        state[(m, h)] = new[m]

    nc.compile()
    return nc


def _get_program():
    if "nc" not in _PROGRAM_CACHE:
        _PROGRAM_CACHE["nc"] = _build_program()
    return _PROGRAM_CACHE["nc"]


def kernel(z_dyn, z_static, dt, U, A_skew_params, gamma_raw, B_ct, C, D_mat=None, **kw):
    # accept the reference's keyword name "D"
    if D_mat is None:
        D_mat = kw.pop("D")
    from concourse import bass_utils

    z_dyn = np.asarray(z_dyn)
    U = np.asarray(U)
    dt_val = float(np.asarray(dt)[0, 0])
    consts = _host_precompute(dt_val, A_skew_params, gamma_raw, B_ct, C, D_mat)

    nc = _get_program()

    in_maps = []
    for c in range(NCORES):
        Uc = U[:, BS * c:BS * (c + 1), :]                        # [256, 64, 32]
        # UALL[32*j + ui, 64*k + b] = U[32k + j, 64c + b, ui]
        uallc = np.ascontiguousarray(
            Uc.reshape(NCH, S, BS, UD).transpose(1, 3, 0, 2).reshape(S * UD, NCH * BS)
        ).astype(np.float32)
        zt0c = np.ascontiguousarray(z_dyn[BS * c:BS * (c + 1), :].T).astype(np.float32)
        m = {"wat": consts["WAT"], "ast": consts["AST"], "gt": consts["GT"],
             "wbt": consts["WBT"], "wct": consts["WCT"], "wdt": consts["WDT"],
             "uall": uallc, "zt0": zt0c}
        in_maps.append(m)

    global LAST_RESULT
    res = bass_utils.run_bass_kernel_spmd(
        nc, in_maps, core_ids=list(range(NCORES)), trace=TRACE,
    )
    LAST_RESULT = res

    Z = np.empty((T, BATCH, D), np.float32)
    Y = np.empty((T, BATCH, NOBS), np.float32)
    for c in range(NCORES):
        zt = res.results[c]["ztout"].reshape(T, D, BS)
        yt = res.results[c]["ytout"].reshape(T, NOBS, BS)
        Z[:, BS * c:BS * (c + 1), :] = zt.transpose(0, 2, 1)
        Y[:, BS * c:BS * (c + 1), :] = yt.transpose(0, 2, 1)
    return Z, Y


# revision 9
# speedup vs baseline: 1.6625x; 1.6625x over previous
# Trainium2 Bass kernel for nn_ConditionedCTKoopmanTransition.
#
# Math (reference): z' = z @ A_bar^T + u @ B_bar^T ; y = z' @ C^T + (u*dt) @ D^T
# scanned over T=256 steps, with A_bar = expm(A_ct*dt), B_bar = A^-1 (A_bar-I) B_ct
# built host-side in float64 from the tiny parameter tensors.
#
# Strategy: data-parallel over batch (8 cores x 64 batch). On each core the
# T=256 sequential scan is restructured into 8 chunks of S=32 steps:
#   F_k   = G @ ublk_k            (G = [A^31 B | ... | B], one K=1024 matmul)
#   a_k+1 = A^32 @ a_k + F_k      (tiny 7-step sequential chain)
#   rollout all 8 chunks batched in the matmul free dim (N=512) for 32 steps
# so nearly all tensor-engine work runs as [K=128, M=128, N=256] fp32r
# matmuls at full (1 cycle/row) speed instead of 256 sequential N=64 steps.
# State is kept d-major (z^T) so each step's PSUM output is directly the
# next step's matmul rhs -- no transposes anywhere on device.

import sys
import numpy as np

sys.path.insert(0, "/opt/trn_rl_repo")

D = 512
UD = 32
NOBS = 50
BATCH = 512
T = 256
NCORES = 8
BS = BATCH // NCORES      # batch shard per core = 64
S = 32                    # chunk length
NCH = T // S              # chunks = 8
HALF = 256                # free-dim half (4 chunks * 64 batch)

_PROGRAM_CACHE = {}
TRACE = False             # test harness can set kernel.TRACE = True
LAST_RESULT = None        # BassKernelResults of the last run (when TRACE)


def _softplus64(x):
    x = np.asarray(x, np.float64)
    return np.log1p(np.exp(-np.abs(x))) + np.maximum(x, 0.0)


def _host_precompute(dt_val, A_skew_params, gamma_raw, B_ct, C, D_mat):
    """float64 host math for the small matrices."""
    import scipy.linalg as sla
    d = D
    A = np.zeros((d, d), np.float64)
    iu = np.triu_indices(d, k=1)
    A[iu] = np.asarray(A_skew_params, np.float64)
    A = A - A.T
    A_ct = A - np.diag(_softplus64(gamma_raw))
    A_bar = sla.expm(A_ct * float(dt_val))
    B_bar = np.linalg.solve(A_ct, (A_bar - np.eye(d)) @ np.asarray(B_ct, np.float64))
    G = np.zeros((d, S * UD), np.float64)
    M = B_bar.copy()
    for j in range(S - 1, -1, -1):
        G[:, j * UD:(j + 1) * UD] = M
        if j > 0:
            M = A_bar @ M
    A_S = np.linalg.matrix_power(A_bar, S)
    f32 = np.float32
    return {
        "WAT": np.ascontiguousarray(A_bar.T.astype(f32)),          # [512, 512] lhsT for A_bar
        "AST": np.ascontiguousarray(A_S.T.astype(f32)),            # [512, 512] lhsT for A^S
        "GT": np.ascontiguousarray(G.T.astype(f32)),               # [1024, 512] lhsT for G
        "WBT": np.ascontiguousarray(np.tile(B_bar.T.astype(f32), (2, 1))),   # [64, 512]
        "WCT": np.ascontiguousarray(np.asarray(C, np.float64).T.astype(f32)),    # [512, 50]
        "WDT": np.ascontiguousarray(np.tile((np.asarray(D_mat, np.float64) * float(dt_val)).T.astype(f32), (2, 1))),  # [64, 50]
    }


def _build_program():
    from concourse import bacc, tile, mybir

    f32 = mybir.dt.float32
    f32r = mybir.dt.float32r

    nc = bacc.Bacc("TRN2", target_bir_lowering=False, debug=False,
                   num_devices=NCORES)

    # DRAM I/O (all fp32 bits; float32r dtype where consumed by matmuls so
    # plain DMAs land in float32r tiles without a casting engine).
    wat_d = nc.dram_tensor("wat", [D, D], f32r, kind="ExternalInput")
    ast_d = nc.dram_tensor("ast", [D, D], f32r, kind="ExternalInput")
    gt_d = nc.dram_tensor("gt", [S * UD, D], f32r, kind="ExternalInput")
    wbt_d = nc.dram_tensor("wbt", [64, D], f32r, kind="ExternalInput")
    wct_d = nc.dram_tensor("wct", [D, NOBS], f32r, kind="ExternalInput")
    wdt_d = nc.dram_tensor("wdt", [64, NOBS], f32r, kind="ExternalInput")
    uall_d = nc.dram_tensor("uall", [S * UD, NCH * BS], f32r, kind="ExternalInput")
    zt0_d = nc.dram_tensor("zt0", [D, BS], f32r, kind="ExternalInput")
    ztout_d = nc.dram_tensor("ztout", [NCH, S, D, BS], f32r, kind="ExternalOutput")
    ytout_d = nc.dram_tensor("ytout", [NCH, S, NOBS, BS], f32, kind="ExternalOutput")

    KT = D // 128   # 4 k-tiles of the d dimension

    with tile.TileContext(nc) as tc:
        with tc.tile_pool(name="const", bufs=1) as cpool, \
             tc.tile_pool(name="fsb", bufs=1) as fpool, \
             tc.tile_pool(name="anp", bufs=1) as anpool, \
             tc.tile_pool(name="st", bufs=2) as stpool, \
             tc.tile_pool(name="ysb", bufs=3) as ypool, \
             tc.tile_pool(name="acc", bufs=5, space="PSUM") as apool, \
             tc.tile_pool(name="yacc", bufs=2, space="PSUM") as yapool:

            # ---- load constants ----
            wa = []
            ast = []
            wc = []
            for kk in range(KT):
                t = cpool.tile([128, D], f32r, tag=f"wa{kk}")
                nc.sync.dma_start(t[:], wat_d.ap()[128 * kk:128 * (kk + 1), :])
                wa.append(t)
                t = cpool.tile([128, D], f32r, tag=f"ast{kk}")
                nc.sync.dma_start(t[:], ast_d.ap()[128 * kk:128 * (kk + 1), :])
                ast.append(t)
                t = cpool.tile([128, NOBS], f32r, tag=f"wc{kk}")
                nc.sync.dma_start(t[:], wct_d.ap()[128 * kk:128 * (kk + 1), :])
                wc.append(t)
            gt = []
            uall = []
            for kk in range(S // 2):   # 16 pair row-tiles of 64 partitions
                t = cpool.tile([64, D], f32r, tag=f"gt{kk}")
                nc.sync.dma_start(t[:], gt_d.ap()[64 * kk:64 * (kk + 1), :])
                gt.append(t)
                t = cpool.tile([64, NCH * BS], f32r, tag=f"u{kk}")
                nc.sync.dma_start(t[:], uall_d.ap()[64 * kk:64 * (kk + 1), :])
                uall.append(t)
            # B/D weights replicated across the 4 partition quarters so the
            # lhsT base_partition matches the u-slice rhs base_partition.
            wb = cpool.tile([64, D], f32r, tag="wb")
            nc.sync.dma_start(wb[:], wbt_d.ap())
            wd = cpool.tile([64, NOBS], f32r, tag="wd")
            nc.sync.dma_start(wd[:], wdt_d.ap())

            def ur_ap(r):
                # u_t^T for step r of every chunk: [32, 512] partition-slice
                q = r % 2
                return uall[r // 2][32 * q:32 * (q + 1), :]

            NF = NCH * BS   # full free dim = 512

            # ---- phase F: F = G @ UBLK   [512, 512] ----
            fsb = {}
            for m in range(KT):
                pf = apool.tile([128, NF], f32, tag="acc")
                for kk in range(S // 2):
                    nc.tensor.matmul(
                        pf[:],
                        gt[kk][:, 128 * m:128 * (m + 1)],
                        uall[kk][:],
                        start=(kk == 0), stop=(kk == S // 2 - 1),
                    )
                t = fpool.tile([128, NF], f32, tag=f"f{m}")
                nc.vector.tensor_copy(t[:], pf[:])
                fsb[m] = t

            # ---- anchor chain: a_{k+1} = A^S a_k + F_k ----
            an = {}
            for m in range(KT):
                an[m] = anpool.tile([128, NF], f32r, tag=f"an{m}",
                                    name=f"an{m}")
            for m in range(KT):
                nc.sync.dma_start(an[m][:, 0:BS],
                                  zt0_d.ap()[128 * m:128 * (m + 1), :])
            for k in range(NCH - 1):
                for m in range(KT):
                    pc = apool.tile([128, BS], f32, tag="acc")
                    for kk in range(KT):
                        nc.tensor.matmul(
                            pc[:],
                            ast[kk][:, 128 * m:128 * (m + 1)],
                            an[kk][:, BS * k:BS * (k + 1)],
                            start=(kk == 0), stop=(kk == KT - 1),
                        )
                    nc.vector.tensor_tensor(
                        an[m][:, BS * (k + 1):BS * (k + 2)],
                        pc[:],
                        fsb[m][:, BS * k:BS * (k + 1)],
                        op=mybir.AluOpType.add,
                    )

            # ---- phase C: batched rollout of all chunks (N=512 matmuls) ----
            state = {m: an[m] for m in range(KT)}
            for r in range(S):
                ur = ur_ap(r)
                q = r % 2
                new = {}
                for m in range(KT):
                    ps = apool.tile([128, NF], f32, tag="acc")
                    for kk in range(KT):
                        nc.tensor.matmul(
                            ps[:],
                            wa[kk][:, 128 * m:128 * (m + 1)],
                            state[kk][:],
                            start=(kk == 0), stop=False,
                        )
                    nc.tensor.matmul(
                        ps[:],
                        wb[32 * q:32 * (q + 1), 128 * m:128 * (m + 1)],
                        ur[:],
                        start=False, stop=True,
                    )
                    ns = stpool.tile([128, NF], f32r, tag=f"st{m}")
                    nc.vector.tensor_copy(ns[:], ps[:])
                    new[m] = ns  # noqa
                    nc.sync.dma_start(
                        ztout_d.ap()[:, r, 128 * m:128 * (m + 1), :]
                        .rearrange("k p e -> p k e"),
                        ns[:].rearrange("p (k e) -> p k e", e=BS),
                    )
                # y = C z' + (dt D) u
                py = yapool.tile([NOBS, NF], f32, tag="yacc")
                for kk in range(KT):
                    nc.tensor.matmul(
                        py[:], wc[kk][:], new[kk][:],
                        start=(kk == 0), stop=False,
                    )
                nc.tensor.matmul(
                    py[:], wd[32 * q:32 * (q + 1), :],
                    ur[:],
                    start=False, stop=True,
                )
                yt = ypool.tile([NOBS, NF], f32, tag="y")
                nc.vector.tensor_copy(yt[:], py[:])
                nc.sync.dma_start(
                    ytout_d.ap()[:, r, :, :].rearrange("k p e -> p k e"),
                    yt[:].rearrange("p (k e) -> p k e", e=BS),
                )
                for m in range(KT):
                    state[m] = new[m]

    nc.compile()
    return nc


def _get_program():
    if "nc" not in _PROGRAM_CACHE:
        _PROGRAM_CACHE["nc"] = _build_program()
    return _PROGRAM_CACHE["nc"]


def kernel(z_dyn, z_static, dt, U, A_skew_params, gamma_raw, B_ct, C, D_mat=None, **kw):
    # accept the reference's keyword name "D"
    if D_mat is None:
        D_mat = kw.pop("D")
    from concourse import bass_utils

    z_dyn = np.asarray(z_dyn)
    U = np.asarray(U)
    dt_val = float(np.asarray(dt)[0, 0])
    consts = _host_precompute(dt_val, A_skew_params, gamma_raw, B_ct, C, D_mat)

    nc = _get_program()

    in_maps = []
    for c in range(NCORES):
        Uc = U[:, BS * c:BS * (c + 1), :]                        # [256, 64, 32]
        # UALL[32*j + ui, 64*k + b] = U[32k + j, 64c + b, ui]
        uallc = np.ascontiguousarray(
            Uc.reshape(NCH, S, BS, UD).transpose(1, 3, 0, 2).reshape(S * UD, NCH * BS)
        ).astype(np.float32)
        zt0c = np.ascontiguousarray(z_dyn[BS * c:BS * (c + 1), :].T).astype(np.float32)
        m = {"wat": consts["WAT"], "ast": consts["AST"], "gt": consts["GT"],
             "wbt": consts["WBT"], "wct": consts["WCT"], "wdt": consts["WDT"],
             "uall": uallc, "zt0": zt0c}
        in_maps.append(m)

    global LAST_RESULT
    res = bass_utils.run_bass_kernel_spmd(
        nc, in_maps, core_ids=list(range(NCORES)), trace=TRACE,
    )
    LAST_RESULT = res

    Z = np.empty((T, BATCH, D), np.float32)
    Y = np.empty((T, BATCH, NOBS), np.float32)
    for c in range(NCORES):
        zt = res.results[c]["ztout"].reshape(T, D, BS)
        yt = res.results[c]["ytout"].reshape(T, NOBS, BS)
        Z[:, BS * c:BS * (c + 1), :] = zt.transpose(0, 2, 1)
        Y[:, BS * c:BS * (c + 1), :] = yt.transpose(0, 2, 1)
    return Z, Y


# revision 10
# speedup vs baseline: 2.0819x; 1.2523x over previous
# Trainium2 Bass kernel for nn_ConditionedCTKoopmanTransition.
#
# Math (reference): z' = z @ A_bar^T + u @ B_bar^T ; y = z' @ C^T + (u*dt) @ D^T
# scanned over T=256 steps, with A_bar = expm(A_ct*dt), B_bar = A^-1 (A_bar-I) B_ct
# built host-side in float64 from the tiny parameter tensors.
#
# Strategy: data-parallel over batch (8 cores x 64 batch). On each core the
# T=256 sequential scan is restructured into 8 chunks of S=32 steps:
#   F_k   = G @ ublk_k            (G = [A^31 B | ... | B], one K=1024 matmul)
#   a_k+1 = A^32 @ a_k + F_k      (tiny 7-step sequential chain)
#   rollout all 8 chunks batched in the matmul free dim (N=512) for 32 steps
# so nearly all tensor-engine work runs as [K=128, M=128, N=256] fp32r
# matmuls at full (1 cycle/row) speed instead of 256 sequential N=64 steps.
# State is kept d-major (z^T) so each step's PSUM output is directly the
# next step's matmul rhs -- no transposes anywhere on device.

import sys
import numpy as np

sys.path.insert(0, "/opt/trn_rl_repo")

D = 512
UD = 32
NOBS = 50
BATCH = 512
T = 256
NCORES = 8
BS = BATCH // NCORES      # batch shard per core = 64
S = 32                    # chunk length
NCH = T // S              # chunks = 8
HALF = 256                # free-dim half (4 chunks * 64 batch)

_PROGRAM_CACHE = {}
TRACE = False             # test harness can set kernel.TRACE = True
LAST_RESULT = None        # BassKernelResults of the last run (when TRACE)
MM_DTYPE = "f16"          # "f16" (fast weight load) or "f32r" (highest precision)


def _softplus64(x):
    x = np.asarray(x, np.float64)
    return np.log1p(np.exp(-np.abs(x))) + np.maximum(x, 0.0)


def _host_precompute(dt_val, A_skew_params, gamma_raw, B_ct, C, D_mat):
    """float64 host math for the small matrices."""
    import scipy.linalg as sla
    d = D
    A = np.zeros((d, d), np.float64)
    iu = np.triu_indices(d, k=1)
    A[iu] = np.asarray(A_skew_params, np.float64)
    A = A - A.T
    A_ct = A - np.diag(_softplus64(gamma_raw))
    A_bar = sla.expm(A_ct * float(dt_val))
    B_bar = np.linalg.solve(A_ct, (A_bar - np.eye(d)) @ np.asarray(B_ct, np.float64))
    G = np.zeros((d, S * UD), np.float64)
    M = B_bar.copy()
    for j in range(S - 1, -1, -1):
        G[:, j * UD:(j + 1) * UD] = M
        if j > 0:
            M = A_bar @ M
    A_S = np.linalg.matrix_power(A_bar, S)
    f32 = np.float32
    return {
        "WAT": np.ascontiguousarray(A_bar.T.astype(f32)),          # [512, 512] lhsT for A_bar
        "AST": np.ascontiguousarray(A_S.T.astype(f32)),            # [512, 512] lhsT for A^S
        "GT": np.ascontiguousarray(G.T.astype(f32)),               # [1024, 512] lhsT for G
        "WBT": np.ascontiguousarray(np.tile(B_bar.T.astype(f32), (2, 1))),   # [64, 512]
        "WCT": np.ascontiguousarray(np.asarray(C, np.float64).T.astype(f32)),    # [512, 50]
        "WDT": np.ascontiguousarray(np.tile((np.asarray(D_mat, np.float64) * float(dt_val)).T.astype(f32), (2, 1))),  # [64, 50]
    }


def _build_program(mm_key):
    from concourse import bacc, tile, mybir

    f32 = mybir.dt.float32
    f32r = {"f16": mybir.dt.float16, "f32r": mybir.dt.float32r}[mm_key]

    nc = bacc.Bacc("TRN2", target_bir_lowering=False, debug=False,
                   num_devices=NCORES)

    # DRAM I/O: matmul-consumed tensors use the matmul dtype so plain DMAs
    # land in matching tiles without a casting engine.
    wat_d = nc.dram_tensor("wat", [D, D], f32r, kind="ExternalInput")
    ast_d = nc.dram_tensor("ast", [D, D], f32r, kind="ExternalInput")
    gt_d = nc.dram_tensor("gt", [S * UD, D], f32r, kind="ExternalInput")
    wbt_d = nc.dram_tensor("wbt", [64, D], f32r, kind="ExternalInput")
    wct_d = nc.dram_tensor("wct", [D, NOBS], f32r, kind="ExternalInput")
    wdt_d = nc.dram_tensor("wdt", [64, NOBS], f32r, kind="ExternalInput")
    uall_d = nc.dram_tensor("uall", [S * UD, NCH * BS], f32r, kind="ExternalInput")
    zt0_d = nc.dram_tensor("zt0", [D, BS], f32r, kind="ExternalInput")
    ztout_d = nc.dram_tensor("ztout", [NCH, S, D, BS], f32r, kind="ExternalOutput")
    # (z output is stored in the matmul dtype; host upcasts)
    ytout_d = nc.dram_tensor("ytout", [NCH, S, NOBS, BS], f32, kind="ExternalOutput")

    KT = D // 128   # 4 k-tiles of the d dimension

    with tile.TileContext(nc) as tc:
        with tc.tile_pool(name="const", bufs=1) as cpool, \
             tc.tile_pool(name="fsb", bufs=1) as fpool, \
             tc.tile_pool(name="anp", bufs=1) as anpool, \
             tc.tile_pool(name="st", bufs=2) as stpool, \
             tc.tile_pool(name="ysb", bufs=3) as ypool, \
             tc.tile_pool(name="acc", bufs=5, space="PSUM") as apool, \
             tc.tile_pool(name="yacc", bufs=2, space="PSUM") as yapool:

            # ---- load constants ----
            wa = []
            ast = []
            wc = []
            for kk in range(KT):
                t = cpool.tile([128, D], f32r, tag=f"wa{kk}")
                nc.sync.dma_start(t[:], wat_d.ap()[128 * kk:128 * (kk + 1), :])
                wa.append(t)
                t = cpool.tile([128, D], f32r, tag=f"ast{kk}")
                nc.sync.dma_start(t[:], ast_d.ap()[128 * kk:128 * (kk + 1), :])
                ast.append(t)
                t = cpool.tile([128, NOBS], f32r, tag=f"wc{kk}")
                nc.sync.dma_start(t[:], wct_d.ap()[128 * kk:128 * (kk + 1), :])
                wc.append(t)
            gt = []
            uall = []
            for kk in range(S // 2):   # 16 pair row-tiles of 64 partitions
                t = cpool.tile([64, D], f32r, tag=f"gt{kk}")
                nc.sync.dma_start(t[:], gt_d.ap()[64 * kk:64 * (kk + 1), :])
                gt.append(t)
                t = cpool.tile([64, NCH * BS], f32r, tag=f"u{kk}")
                nc.sync.dma_start(t[:], uall_d.ap()[64 * kk:64 * (kk + 1), :])
                uall.append(t)
            # B/D weights replicated across the 4 partition quarters so the
            # lhsT base_partition matches the u-slice rhs base_partition.
            wb = cpool.tile([64, D], f32r, tag="wb")
            nc.sync.dma_start(wb[:], wbt_d.ap())
            wd = cpool.tile([64, NOBS], f32r, tag="wd")
            nc.sync.dma_start(wd[:], wdt_d.ap())

            def ur_ap(r):
                # u_t^T for step r of every chunk: [32, 512] partition-slice
                q = r % 2
                return uall[r // 2][32 * q:32 * (q + 1), :]

            NF = NCH * BS   # full free dim = 512

            # ---- phase F: F = G @ UBLK   [512, 512] ----
            fsb = {}
            for m in range(KT):
                pf = apool.tile([128, NF], f32, tag="acc")
                for kk in range(S // 2):
                    nc.tensor.matmul(
                        pf[:],
                        gt[kk][:, 128 * m:128 * (m + 1)],
                        uall[kk][:],
                        start=(kk == 0), stop=(kk == S // 2 - 1),
                    )
                t = fpool.tile([128, NF], f32, tag=f"f{m}")
                nc.vector.tensor_copy(t[:], pf[:])
                fsb[m] = t

            # ---- anchor chain: a_{k+1} = A^S a_k + F_k ----
            an = {}
            for m in range(KT):
                an[m] = anpool.tile([128, NF], f32r, tag=f"an{m}",
                                    name=f"an{m}")
            for m in range(KT):
                nc.sync.dma_start(an[m][:, 0:BS],
                                  zt0_d.ap()[128 * m:128 * (m + 1), :])
            for k in range(NCH - 1):
                for m in range(KT):
                    pc = apool.tile([128, BS], f32, tag="acc")
                    for kk in range(KT):
                        nc.tensor.matmul(
                            pc[:],
                            ast[kk][:, 128 * m:128 * (m + 1)],
                            an[kk][:, BS * k:BS * (k + 1)],
                            start=(kk == 0), stop=(kk == KT - 1),
                        )
                    nc.vector.tensor_tensor(
                        an[m][:, BS * (k + 1):BS * (k + 2)],
                        pc[:],
                        fsb[m][:, BS * k:BS * (k + 1)],
                        op=mybir.AluOpType.add,
                    )

            # ---- phase C: batched rollout of all chunks (N=512 matmuls) ----
            state = {m: an[m] for m in range(KT)}
            for r in range(S):
                ur = ur_ap(r)
                q = r % 2
                new = {}
                for m in range(KT):
                    ps = apool.tile([128, NF], f32, tag="acc")
                    for kk in range(KT):
                        nc.tensor.matmul(
                            ps[:],
                            wa[kk][:, 128 * m:128 * (m + 1)],
                            state[kk][:],
                            start=(kk == 0), stop=False,
                        )
                    nc.tensor.matmul(
                        ps[:],
                        wb[32 * q:32 * (q + 1), 128 * m:128 * (m + 1)],
                        ur[:],
                        start=False, stop=True,
                    )
                    ns = stpool.tile([128, NF], f32r, tag=f"st{m}")
                    nc.vector.tensor_copy(ns[:], ps[:])
                    new[m] = ns  # noqa
                    nc.sync.dma_start(
                        ztout_d.ap()[:, r, 128 * m:128 * (m + 1), :]
                        .rearrange("k p e -> p k e"),
                        ns[:].rearrange("p (k e) -> p k e", e=BS),
                    )
                # y = C z' + (dt D) u
                py = yapool.tile([NOBS, NF], f32, tag="yacc")
                for kk in range(KT):
                    nc.tensor.matmul(
                        py[:], wc[kk][:], new[kk][:],
                        start=(kk == 0), stop=False,
                    )
                nc.tensor.matmul(
                    py[:], wd[32 * q:32 * (q + 1), :],
                    ur[:],
                    start=False, stop=True,
                )
                yt = ypool.tile([NOBS, NF], f32, tag="y")
                nc.vector.tensor_copy(yt[:], py[:])
                nc.sync.dma_start(
                    ytout_d.ap()[:, r, :, :].rearrange("k p e -> p k e"),
                    yt[:].rearrange("p (k e) -> p k e", e=BS),
                )
                for m in range(KT):
                    state[m] = new[m]

    nc.compile()
    return nc


def _get_program():
    if MM_DTYPE not in _PROGRAM_CACHE:
        _PROGRAM_CACHE[MM_DTYPE] = _build_program(MM_DTYPE)
    return _PROGRAM_CACHE[MM_DTYPE]


def kernel(z_dyn, z_static, dt, U, A_skew_params, gamma_raw, B_ct, C, D_mat=None, **kw):
    # accept the reference's keyword name "D"
    if D_mat is None:
        D_mat = kw.pop("D")
    from concourse import bass_utils

    z_dyn = np.asarray(z_dyn)
    U = np.asarray(U)
    dt_val = float(np.asarray(dt)[0, 0])
    consts = _host_precompute(dt_val, A_skew_params, gamma_raw, B_ct, C, D_mat)

    nc = _get_program()

    mmnp = np.float16 if MM_DTYPE == "f16" else np.float32
    wat = consts["WAT"].astype(mmnp)
    astc = consts["AST"].astype(mmnp)
    gtc = consts["GT"].astype(mmnp)
    wbt = consts["WBT"].astype(mmnp)
    wct = consts["WCT"].astype(mmnp)
    wdt = consts["WDT"].astype(mmnp)

    in_maps = []
    for c in range(NCORES):
        Uc = U[:, BS * c:BS * (c + 1), :]                        # [256, 64, 32]
        # UALL[32*j + ui, 64*k + b] = U[32k + j, 64c + b, ui]
        uallc = np.ascontiguousarray(
            Uc.reshape(NCH, S, BS, UD).transpose(1, 3, 0, 2).reshape(S * UD, NCH * BS)
        ).astype(mmnp)
        zt0c = np.ascontiguousarray(z_dyn[BS * c:BS * (c + 1), :].T).astype(mmnp)
        m = {"wat": wat, "ast": astc, "gt": gtc,
             "wbt": wbt, "wct": wct, "wdt": wdt,
             "uall": uallc, "zt0": zt0c}
        in_maps.append(m)

    global LAST_RESULT
    res = bass_utils.run_bass_kernel_spmd(
        nc, in_maps, core_ids=list(range(NCORES)), trace=TRACE,
    )
    LAST_RESULT = res

    Z = np.empty((T, BATCH, D), np.float32)
    Y = np.empty((T, BATCH, NOBS), np.float32)
    for c in range(NCORES):
        zt = res.results[c]["ztout"].astype(np.float32).reshape(T, D, BS)
        yt = res.results[c]["ytout"].reshape(T, NOBS, BS)
        Z[:, BS * c:BS * (c + 1), :] = zt.transpose(0, 2, 1)
        Y[:, BS * c:BS * (c + 1), :] = yt.transpose(0, 2, 1)
    return Z, Y


# revision 11
# speedup vs baseline: 2.1513x; 1.0333x over previous
# Trainium2 Bass kernel for nn_ConditionedCTKoopmanTransition.
#
# Math (reference): z' = z @ A_bar^T + u @ B_bar^T ; y = z' @ C^T + (u*dt) @ D^T
# scanned over T=256 steps, with A_bar = expm(A_ct*dt), B_bar = A^-1 (A_bar-I) B_ct
# built host-side in float64 from the tiny parameter tensors.
#
# Strategy: data-parallel over batch (8 cores x 64 batch). On each core the
# T=256 sequential scan is restructured into 8 chunks of S=32 steps:
#   F_k   = G @ ublk_k            (G = [A^31 B | ... | B], one K=1024 matmul)
#   a_k+1 = A^32 @ a_k + F_k      (tiny 7-step sequential chain)
#   rollout all 8 chunks batched in the matmul free dim (N=512) for 32 steps
# so nearly all tensor-engine work runs as [K=128, M=128, N=256] fp32r
# matmuls at full (1 cycle/row) speed instead of 256 sequential N=64 steps.
# State is kept d-major (z^T) so each step's PSUM output is directly the
# next step's matmul rhs -- no transposes anywhere on device.

import sys
import numpy as np

sys.path.insert(0, "/opt/trn_rl_repo")

D = 512
UD = 32
NOBS = 50
BATCH = 512
T = 256
NCORES = 8
BS = BATCH // NCORES      # batch shard per core = 64
S = 32                    # chunk length
NCH = T // S              # chunks = 8
HALF = 256                # free-dim half (4 chunks * 64 batch)

_PROGRAM_CACHE = {}
TRACE = False             # test harness can set kernel.TRACE = True
LAST_RESULT = None        # BassKernelResults of the last run (when TRACE)
MM_DTYPE = "f16"          # "f16" (fast weight load) or "f32r" (highest precision)


def _softplus64(x):
    x = np.asarray(x, np.float64)
    return np.log1p(np.exp(-np.abs(x))) + np.maximum(x, 0.0)


def _host_precompute(dt_val, A_skew_params, gamma_raw, B_ct, C, D_mat):
    """float64 host math for the small matrices."""
    import scipy.linalg as sla
    d = D
    A = np.zeros((d, d), np.float64)
    iu = np.triu_indices(d, k=1)
    A[iu] = np.asarray(A_skew_params, np.float64)
    A = A - A.T
    A_ct = A - np.diag(_softplus64(gamma_raw))
    A_bar = sla.expm(A_ct * float(dt_val))
    B_bar = np.linalg.solve(A_ct, (A_bar - np.eye(d)) @ np.asarray(B_ct, np.float64))
    G = np.zeros((d, S * UD), np.float64)
    M = B_bar.copy()
    for j in range(S - 1, -1, -1):
        G[:, j * UD:(j + 1) * UD] = M
        if j > 0:
            M = A_bar @ M
    A_S = np.linalg.matrix_power(A_bar, S)
    f32 = np.float32
    return {
        "WAT": np.ascontiguousarray(A_bar.T.astype(f32)),          # [512, 512] lhsT for A_bar
        "AST": np.ascontiguousarray(A_S.T.astype(f32)),            # [512, 512] lhsT for A^S
        "GT": np.ascontiguousarray(G.T.astype(f32)),               # [1024, 512] lhsT for G
        "WBT": np.ascontiguousarray(np.tile(B_bar.T.astype(f32), (2, 1))),   # [64, 512]
        "WCT": np.ascontiguousarray(np.asarray(C, np.float64).T.astype(f32)),    # [512, 50]
        "WDT": np.ascontiguousarray(np.tile((np.asarray(D_mat, np.float64) * float(dt_val)).T.astype(f32), (2, 1))),  # [64, 50]
    }


def _build_program(mm_key):
    from concourse import bacc, tile, mybir

    f32 = mybir.dt.float32
    f32r = {"f16": mybir.dt.float16, "f32r": mybir.dt.float32r}[mm_key]

    nc = bacc.Bacc("TRN2", target_bir_lowering=False, debug=False,
                   num_devices=NCORES)

    # DRAM I/O: matmul-consumed tensors use the matmul dtype so plain DMAs
    # land in matching tiles without a casting engine.
    wat_d = nc.dram_tensor("wat", [D, D], f32r, kind="ExternalInput")
    ast_d = nc.dram_tensor("ast", [D, D], f32r, kind="ExternalInput")
    gt_d = nc.dram_tensor("gt", [S * UD, D], f32r, kind="ExternalInput")
    wbt_d = nc.dram_tensor("wbt", [64, D], f32r, kind="ExternalInput")
    wct_d = nc.dram_tensor("wct", [D, NOBS], f32r, kind="ExternalInput")
    wdt_d = nc.dram_tensor("wdt", [64, NOBS], f32r, kind="ExternalInput")
    uall_d = nc.dram_tensor("uall", [S * UD, NCH * BS], f32r, kind="ExternalInput")
    zt0_d = nc.dram_tensor("zt0", [D, BS], f32r, kind="ExternalInput")
    ztout_d = nc.dram_tensor("ztout", [NCH, S, D, BS], f32r, kind="ExternalOutput")
    # (z output is stored in the matmul dtype; host upcasts)
    ytout_d = nc.dram_tensor("ytout", [NCH, S, NOBS, BS], f32, kind="ExternalOutput")

    KT = D // 128   # 4 k-tiles of the d dimension

    with tile.TileContext(nc) as tc:
        with tc.tile_pool(name="const", bufs=1) as cpool, \
             tc.tile_pool(name="fsb", bufs=1) as fpool, \
             tc.tile_pool(name="anp", bufs=1) as anpool, \
             tc.tile_pool(name="st", bufs=2) as stpool, \
             tc.tile_pool(name="ysb", bufs=3) as ypool, \
             tc.tile_pool(name="acc", bufs=6, space="PSUM") as apool, \
             tc.tile_pool(name="yacc", bufs=2, space="PSUM") as yapool:

            # ---- load constants (phase-F inputs first so PE starts early) ----
            gt = []
            ublk = []
            for kk in range(S * UD // 128):   # 8 K=128 row-tiles for phase F
                t = cpool.tile([128, D], f32r, tag=f"gt{kk}")
                nc.sync.dma_start(t[:], gt_d.ap()[128 * kk:128 * (kk + 1), :])
                gt.append(t)
                t = cpool.tile([128, NCH * BS], f32r, tag=f"ub{kk}")
                nc.sync.dma_start(t[:], uall_d.ap()[128 * kk:128 * (kk + 1), :])
                ublk.append(t)
            wa = []
            ast = []
            wc = []
            for kk in range(KT):
                t = cpool.tile([128, D], f32r, tag=f"wa{kk}")
                nc.sync.dma_start(t[:], wat_d.ap()[128 * kk:128 * (kk + 1), :])
                wa.append(t)
                t = cpool.tile([128, D], f32r, tag=f"ast{kk}")
                nc.sync.dma_start(t[:], ast_d.ap()[128 * kk:128 * (kk + 1), :])
                ast.append(t)
                t = cpool.tile([128, NOBS], f32r, tag=f"wc{kk}")
                nc.sync.dma_start(t[:], wct_d.ap()[128 * kk:128 * (kk + 1), :])
                wc.append(t)
            # second copy of u in 64-partition pair tiles for per-step slices
            uall = []
            for kk in range(S // 2):
                t = cpool.tile([64, NCH * BS], f32r, tag=f"u{kk}")
                nc.sync.dma_start(t[:], uall_d.ap()[64 * kk:64 * (kk + 1), :])
                uall.append(t)
            # B/D weights replicated across partition halves so the lhsT
            # base_partition matches the u-slice rhs base_partition.
            wb = cpool.tile([64, D], f32r, tag="wb")
            nc.sync.dma_start(wb[:], wbt_d.ap())
            wd = cpool.tile([64, NOBS], f32r, tag="wd")
            nc.sync.dma_start(wd[:], wdt_d.ap())

            def ur_ap(r):
                # u_t^T for step r of every chunk: [32, 512] partition-slice
                q = r % 2
                return uall[r // 2][32 * q:32 * (q + 1), :]

            NF = NCH * BS   # full free dim = 512

            # ---- phase F: F = G @ UBLK   [512, 512] ----
            fsb = {}
            NGT = S * UD // 128
            for m in range(KT):
                pf = apool.tile([128, NF], f32, tag="acc")
                for kk in range(NGT):
                    nc.tensor.matmul(
                        pf[:],
                        gt[kk][:, 128 * m:128 * (m + 1)],
                        ublk[kk][:],
                        start=(kk == 0), stop=(kk == NGT - 1),
                    )
                t = fpool.tile([128, NF], f32, tag=f"f{m}")
                nc.vector.tensor_copy(t[:], pf[:])
                fsb[m] = t

            # ---- anchor chain: a_{k+1} = A^S a_k + F_k ----
            an = {}
            for m in range(KT):
                an[m] = anpool.tile([128, NF], f32r, tag=f"an{m}",
                                    name=f"an{m}")
            for m in range(KT):
                nc.sync.dma_start(an[m][:, 0:BS],
                                  zt0_d.ap()[128 * m:128 * (m + 1), :])
            for k in range(NCH - 1):
                for m in range(KT):
                    pc = apool.tile([128, BS], f32, tag="acc")
                    for kk in range(KT):
                        nc.tensor.matmul(
                            pc[:],
                            ast[kk][:, 128 * m:128 * (m + 1)],
                            an[kk][:, BS * k:BS * (k + 1)],
                            start=(kk == 0), stop=(kk == KT - 1),
                        )
                    nc.vector.tensor_tensor(
                        an[m][:, BS * (k + 1):BS * (k + 2)],
                        pc[:],
                        fsb[m][:, BS * k:BS * (k + 1)],
                        op=mybir.AluOpType.add,
                    )

            # ---- phase C: batched rollout of all chunks (N=512 matmuls) ----
            state = {m: an[m] for m in range(KT)}
            for r in range(S):
                ur = ur_ap(r)
                q = r % 2
                new = {}
                for m in range(KT):
                    ps = apool.tile([128, NF], f32, tag="acc")
                    for kk in range(KT):
                        nc.tensor.matmul(
                            ps[:],
                            wa[kk][:, 128 * m:128 * (m + 1)],
                            state[kk][:],
                            start=(kk == 0), stop=False,
                        )
                    nc.tensor.matmul(
                        ps[:],
                        wb[32 * q:32 * (q + 1), 128 * m:128 * (m + 1)],
                        ur[:],
                        start=False, stop=True,
                    )
                    ns = stpool.tile([128, NF], f32r, tag=f"st{m}")
                    nc.vector.tensor_copy(ns[:], ps[:])
                    new[m] = ns  # noqa
                    nc.sync.dma_start(
                        ztout_d.ap()[:, r, 128 * m:128 * (m + 1), :]
                        .rearrange("k p e -> p k e"),
                        ns[:].rearrange("p (k e) -> p k e", e=BS),
                    )
                # y = C z' + (dt D) u
                py = yapool.tile([NOBS, NF], f32, tag="yacc")
                for kk in range(KT):
                    nc.tensor.matmul(
                        py[:], wc[kk][:], new[kk][:],
                        start=(kk == 0), stop=False,
                    )
                nc.tensor.matmul(
                    py[:], wd[32 * q:32 * (q + 1), :],
                    ur[:],
                    start=False, stop=True,
                )
                yt = ypool.tile([NOBS, NF], f32, tag="y")
                nc.vector.tensor_copy(yt[:], py[:])
                nc.sync.dma_start(
                    ytout_d.ap()[:, r, :, :].rearrange("k p e -> p k e"),
                    yt[:].rearrange("p (k e) -> p k e", e=BS),
                )
                for m in range(KT):
                    state[m] = new[m]

    nc.compile()
    return nc


def _get_program():
    if MM_DTYPE not in _PROGRAM_CACHE:
        _PROGRAM_CACHE[MM_DTYPE] = _build_program(MM_DTYPE)
    return _PROGRAM_CACHE[MM_DTYPE]


def kernel(z_dyn, z_static, dt, U, A_skew_params, gamma_raw, B_ct, C, D_mat=None, **kw):
    # accept the reference's keyword name "D"
    if D_mat is None:
        D_mat = kw.pop("D")
    from concourse import bass_utils

    z_dyn = np.asarray(z_dyn)
    U = np.asarray(U)
    dt_val = float(np.asarray(dt)[0, 0])
    consts = _host_precompute(dt_val, A_skew_params, gamma_raw, B_ct, C, D_mat)

    nc = _get_program()

    mmnp = np.float16 if MM_DTYPE == "f16" else np.float32
    wat = consts["WAT"].astype(mmnp)
    astc = consts["AST"].astype(mmnp)
    gtc = consts["GT"].astype(mmnp)
    wbt = consts["WBT"].astype(mmnp)
    wct = consts["WCT"].astype(mmnp)
    wdt = consts["WDT"].astype(mmnp)

    in_maps = []
    for c in range(NCORES):
        Uc = U[:, BS * c:BS * (c + 1), :]                        # [256, 64, 32]
        # UALL[32*j + ui, 64*k + b] = U[32k + j, 64c + b, ui]
        uallc = np.ascontiguousarray(
            Uc.reshape(NCH, S, BS, UD).transpose(1, 3, 0, 2).reshape(S * UD, NCH * BS)
        ).astype(mmnp)
        zt0c = np.ascontiguousarray(z_dyn[BS * c:BS * (c + 1), :].T).astype(mmnp)
        m = {"wat": wat, "ast": astc, "gt": gtc,
             "wbt": wbt, "wct": wct, "wdt": wdt,
             "uall": uallc, "zt0": zt0c}
        in_maps.append(m)

    global LAST_RESULT
    res = bass_utils.run_bass_kernel_spmd(
        nc, in_maps, core_ids=list(range(NCORES)), trace=TRACE,
    )
    LAST_RESULT = res

    Z = np.empty((T, BATCH, D), np.float32)
    Y = np.empty((T, BATCH, NOBS), np.float32)
    for c in range(NCORES):
        zt = res.results[c]["ztout"].astype(np.float32).reshape(T, D, BS)
        yt = res.results[c]["ytout"].reshape(T, NOBS, BS)
        Z[:, BS * c:BS * (c + 1), :] = zt.transpose(0, 2, 1)
        Y[:, BS * c:BS * (c + 1), :] = yt.transpose(0, 2, 1)
    return Z, Y
